# revision 8
# baseline (speedup 1.0000x reference)
"""Trainium2 Bass kernel for nn_DyConv (MoE routed dynamic conv).

Model (per batch image b):
  g = mean(x[b], spatial)                      # [C]
  w = softmax(fc2(relu(fc1(g))))               # [E]  router weights
  out[b] = sum_e w[e] * silu(bn_e(conv3x3_e(x[b])))

Strategy: pure data-parallel over batch. B=16 images / 8 cores = 2 images
per core; router + experts replicated. No collectives.

Per-core device program (per image):
  - x arrives host-padded to a flat 162x162 zero-padded layout in bf16.
  - Two SBUF "region" copies per half-image stack 2 shifted taps on the
    128 partitions: A = [x(+0); x(+1)], B = [x(+2); x(+164)].  A K=128
    matmul against A at offset o contracts taps (o, o+1) for all 64
    channels; 9 conv taps = 5 matmuls (3xA-pair, 1xB-pair, 1 half-K
    single) per expert-pair.  Two expert pairs stacked in M=128.
  - BN scale folded into conv weights on host; BN shift applied as the
    per-partition bias of the SiLU activation (ScalarE, PSUM->SBUF bf16).
  - Router: DVE chunk-reduces over the bf16 image; tiny matmuls + a
    tanh-based exp for the 4-way softmax; the mixing weights become two
    scaled-identity bf16 [128,64] lhsT tiles built on ScalarE.  The
    router is emitted in 4 stages one conv-tile apart so the in-order
    PE queue never waits on the ACT/DVE round trips, and image s+1's
    router is computed in the middle of image s.
  - Mix: 2 accumulating matmuls -> PSUM [64,N]; DVE compacting copy
    (drops the 2 pad cols) into a 3-tile (9-row) bounce group; one
    contiguous DMA per group to the output.
  - Startup: all consts ride 2 packed DMAs; regA(h0) chunks on the
    gpsimd queue and regB(h0) chunks on the scalar queue land first so
    the conv stream saturates from ~11us; later regions are 2-chunk
    transfers prefetched behind the compute.
"""
import os
import sys
import numpy as np

if "/opt/trn_rl_repo" not in sys.path:
    sys.path.insert(0, "/opt/trn_rl_repo")

import ml_dtypes  # noqa: E402

BF16_NP = ml_dtypes.bfloat16

B, C, H, W = 16, 64, 160, 160
E, R = 4, 16
NCORES = 8
IMG_PER_CORE = B // NCORES          # 2
WP = W + 2                          # 162 padded row
LP = (H + 2) * WP                   # 26244 padded flat image
LHOST = 26600                       # host buffer with zero margin (max read 26408)
RLEN = 82 * WP                      # 13284: half-image region (80 out rows + 2 halo)
HB = 80 * WP                        # 12960: out-grid columns per half
NT = 486                            # psum tile = 3 out rows
BN_EPS = 1e-3

# chunk boundaries within a half-image region
CH4 = [0, 3321, 6642, 9963, 13284]      # startup loads: fine-grained
CH2 = [0, 6642, 13284]                  # steady-state prefetch loads
# router reduce boundaries: h0 covers [0, HB), h1 covers [0, RLEN)
REDB0 = [0, 3321, 6642, 9963, 12960]

_CACHE = {}


def _build_program(reps=1):
    import concourse.bacc as bacc
    import concourse.tile as tile
    from concourse import mybir

    BF16 = mybir.dt.bfloat16
    F32 = mybir.dt.float32
    AF = mybir.ActivationFunctionType
    ALU = mybir.AluOpType
    AX = mybir.AxisListType

    nc = bacc.Bacc("TRN2", target_bir_lowering=False, debug=False,
                   num_devices=NCORES)

    xp_d = nc.dram_tensor("xp", [IMG_PER_CORE, 4, C, LHOST], BF16, kind="ExternalInput")
    cbf_d = nc.dram_tensor("cbf", [128, 1300], BF16, kind="ExternalInput")
    cf32_d = nc.dram_tensor("cf32", [128, 131], F32, kind="ExternalInput")
    out_d = nc.dram_tensor("out", [IMG_PER_CORE, C, H, W], F32, kind="ExternalOutput")

    seq = [i % IMG_PER_CORE for i in range(IMG_PER_CORE * reps)]
    S = len(seq)

    with tile.TileContext(nc) as tc:
        with tc.tile_pool(name="consts", bufs=1) as cp, \
             tc.tile_pool(name="regs", bufs=2) as rp, \
             tc.tile_pool(name="work", bufs=2) as wp, \
             tc.tile_pool(name="bounce", bufs=2) as bp, \
             tc.tile_pool(name="psum", bufs=1, space="PSUM") as pp:

            cbf = cp.tile([128, 1300], BF16)
            nc.sync.dma_start(cbf[:, :], cbf_d[:, :])
            cf32 = cp.tile([128, 131], F32)
            nc.sync.dma_start(cf32[:, :], cf32_d[:, :])
            o64_sb = cp.tile([1, 64], F32)
            nc.gpsimd.memset(o64_sb[:, :], 1.0)

            wk_sb = cbf[:, 0:1024]
            wks_sb = cbf[:, 1024:1280]
            fc1t_sb = cbf[0:64, 1280:1296]
            fc2t_sb = cbf[0:16, 1296:1300]
            idc_sb = cf32[:, 0:128]
            bnb_sb = cf32[:, 128:130]
            fc2b_sb = cf32[0:4, 130:131]

            regA_t = {}
            regB_t = {}
            parts_t = {}
            mixw_of = {}
            routst = {}

            def disp_A(s, h, eng, chb):
                img = seq[s]
                t = rp.tile([128, RLEN], BF16, tag="regA", name=f"regA{s}h{h}",
                            bufs=3)
                regA_t[(s, h)] = t
                for c in range(len(chb) - 1):
                    eng.dma_start(
                        t[:, chb[c]:chb[c + 1]],
                        xp_d[img, 0:2, :, h * HB + chb[c]:h * HB + chb[c + 1]]
                        .rearrange("j c f -> (j c) f"))

            def disp_B(s, h, eng, chb):
                img = seq[s]
                t = rp.tile([128, RLEN], BF16, tag="regB", name=f"regB{s}h{h}",
                            bufs=2)
                regB_t[(s, h)] = t
                for c in range(len(chb) - 1):
                    eng.dma_start(
                        t[:, chb[c]:chb[c + 1]],
                        xp_d[img, 2:4, :, h * HB + chb[c]:h * HB + chb[c + 1]]
                        .rearrange("j c f -> (j c) f"))

            def emit_reduce_chunk(s, h, c):
                # partial router sum over one column chunk (DVE)
                if s not in parts_t:
                    parts_t[s] = wp.tile([64, 8], F32, tag="parts", bufs=2,
                                         name=f"parts{s}")
                bounds = REDB0 if h == 0 else CH4
                nc.vector.tensor_reduce(
                    parts_t[s][:, h * 4 + c:h * 4 + c + 1],
                    regA_t[(s, h)][0:64, bounds[c]:bounds[c + 1]],
                    axis=AX.X, op=ALU.add)

            def emit_router_stage(s, k):
                st = routst.setdefault(s, {})
                if k == 0:
                    st["gsum"] = wp.tile([64, 1], F32, tag="gsum", name=f"gsum{s}")
                    nc.vector.tensor_reduce(st["gsum"][:, :], parts_t[s][:, 0:8],
                                            axis=AX.X, op=ALU.add)
                    st["gbf"] = wp.tile([64, 1], BF16, tag="gbf", name=f"gbf{s}")
                    nc.vector.tensor_copy(st["gbf"][:, :], st["gsum"][:, :])
                    h_ps = pp.tile([16, 1], F32, tag="pr", name=f"hps{s}")
                    nc.tensor.matmul(h_ps[:, :], fc1t_sb, st["gbf"][:, :],
                                     start=True, stop=True)
                    st["hbf"] = wp.tile([16, 1], BF16, tag="hbf", name=f"hbf{s}")
                    nc.scalar.activation(st["hbf"][:, :], h_ps[:, :], AF.Relu)
                elif k == 1:
                    l_ps = pp.tile([4, 1], F32, tag="pr", name=f"lps{s}")
                    nc.tensor.matmul(l_ps[:, :], fc2t_sb, st["hbf"][:, :],
                                     start=True, stop=True)
                    st["lsb"] = wp.tile([4, 1], F32, tag="lsb", name=f"lsb{s}")
                    nc.scalar.activation(st["lsb"][:, :], l_ps[:, :], AF.Identity,
                                         bias=fc2b_sb)
                elif k == 2:
                    lrow_ps = pp.tile([1, 4], F32, tag="pr", name=f"lrow{s}")
                    nc.tensor.transpose(lrow_ps[:, :], st["lsb"][:, :], idc_sb[0:4, 0:4])
                    # exp(l) = (1 + tanh(l/2)) / (1 - tanh(l/2)); logits are O(0.5)
                    trow = wp.tile([1, 4], F32, tag="trow", name=f"trow{s}")
                    nc.scalar.activation(trow[:, :], lrow_ps[:, :], AF.Tanh, scale=0.5)
                    num = wp.tile([1, 4], F32, tag="num", name=f"num{s}")
                    nc.vector.tensor_scalar_add(num[:, :], trow[:, :], 1.0)
                    den = wp.tile([1, 4], F32, tag="den", name=f"den{s}")
                    nc.vector.tensor_scalar(den[:, :], trow[:, :], -1.0, 1.0,
                                            op0=ALU.mult, op1=ALU.add)
                    rec = wp.tile([1, 4], F32, tag="rec", name=f"rec{s}")
                    nc.vector.reciprocal(rec[:, :], den[:, :])
                    erow = wp.tile([1, 4], F32, tag="erow", name=f"erow{s}")
                    nc.vector.tensor_tensor(erow[:, :], num[:, :], rec[:, :], op=ALU.mult)
                    ssum = wp.tile([1, 1], F32, tag="ssum", name=f"ssum{s}")
                    nc.vector.tensor_reduce(ssum[:, :], erow[:, :], axis=AX.X, op=ALU.add)
                    sinv = wp.tile([1, 1], F32, tag="sinv", name=f"sinv{s}")
                    nc.vector.reciprocal(sinv[:, :], ssum[:, :])
                    wrow = wp.tile([1, 4], F32, tag="wrow", name=f"wrow{s}")
                    nc.vector.tensor_scalar_mul(wrow[:, :], erow[:, :], sinv[:, 0:1])
                    rowA = wp.tile([1, 128], F32, tag="rowA", name=f"rowA{s}")
                    nc.vector.tensor_scalar_mul(rowA[:, 0:64], o64_sb[:, :], wrow[:, 0:1])
                    nc.vector.tensor_scalar_mul(rowA[:, 64:128], o64_sb[:, :], wrow[:, 1:2])
                    rowB = wp.tile([1, 128], F32, tag="rowB", name=f"rowB{s}")
                    nc.vector.tensor_scalar_mul(rowB[:, 0:64], o64_sb[:, :], wrow[:, 2:3])
                    nc.vector.tensor_scalar_mul(rowB[:, 64:128], o64_sb[:, :], wrow[:, 3:4])
                    st["rowA"], st["rowB"] = rowA, rowB
                else:
                    wc_ps = pp.tile([128, 2], F32, tag="pr", name=f"wc{s}")
                    nc.tensor.matmul(wc_ps[:, 0:1], st["rowA"][:, :], idc_sb[0:1, 0:1],
                                     start=True, stop=True)
                    nc.tensor.matmul(wc_ps[:, 1:2], st["rowB"][:, :], idc_sb[0:1, 0:1],
                                     start=True, stop=True)
                    wcol = wp.tile([128, 2], F32, tag="wcol", name=f"wcol{s}")
                    nc.scalar.copy(wcol[:, :], wc_ps[:, :])
                    mA = wp.tile([128, 128], BF16, tag="mixA", name=f"mixA{s}")
                    nc.scalar.activation(mA[:, :], idc_sb, AF.Copy, scale=wcol[:, 0:1])
                    mB = wp.tile([128, 128], BF16, tag="mixB", name=f"mixB{s}")
                    nc.scalar.activation(mB[:, :], idc_sb, AF.Copy, scale=wcol[:, 1:2])
                    mixw_of[s] = (mA, mB)

            # ---- mix + compacting copy + grouped output DMA ----
            grp_state = {}

            def emit_mix(st):
                tsbs, N, h, t, s = st
                img = seq[s]
                mA, mB = mixw_of[s]
                po = pp.tile([128, N], F32, tag="po", bufs=3)
                nc.tensor.matmul(po[:, :], mA[:, :], tsbs[0][:, :],
                                 start=True, stop=False)
                nc.tensor.matmul(po[:, :], mB[:, :], tsbs[1][:, :],
                                 start=False, stop=True)
                g, j = divmod(t, 3)
                if j == 0:
                    grp_state["t"] = bp.tile([64, 1440], F32, tag="bounce",
                                             name="bounce")
                bt = grp_state["t"]
                nrows = 3 if t < 26 else 2
                src = po[0:64, :].rearrange("p (r c) -> p r c", c=WP)[:, 0:nrows, 0:W]
                dst = bt[:, j * 480:j * 480 + nrows * W].rearrange(
                    "p (r c) -> p r c", c=W)
                nc.vector.tensor_copy(dst, src)
                if j == 2 or t == 26:
                    r0 = h * 80 + g * 9
                    nr = 9 if g < 8 else 8
                    nc.sync.dma_start(
                        out_d[img, :, r0:r0 + nr, :],
                        bt[:, 0:nr * W].rearrange("p (r c) -> p r c", c=W))

            # ---- main loop over image sequence ----
            pending = []
            for s in range(S):
                if s == 0:
                    disp_A(0, 0, nc.gpsimd, CH4)
                    disp_B(0, 0, nc.scalar, CH4)
                    disp_A(0, 1, nc.gpsimd, CH4)
                    disp_B(0, 1, nc.scalar, CH4)
                    for hh in (0, 1):
                        for cc in range(4):
                            emit_reduce_chunk(0, hh, cc)
                else:
                    # h1's B region: its buffer frees at image s-1's end
                    disp_B(s, 1, nc.sync, CH2)
                if s + 1 < S:
                    disp_A(s + 1, 0, nc.gpsimd, CH2)
                ROUTER_G = 10 if s == 0 else -10   # stage-0 gtile of this image's router
                for h in (0, 1):
                    regA = regA_t[(s, h)]
                    regB = regB_t[(s, h)]
                    for t in range(27):
                        gtile = h * 27 + t
                        if s + 1 < S:
                            if gtile == 28:
                                disp_B(s + 1, 0, nc.sync, CH2)
                            elif gtile == 30:
                                disp_A(s + 1, 1, nc.scalar, CH2)
                            elif 38 <= gtile <= 41:
                                emit_reduce_chunk(s + 1, 0, gtile - 38)
                            elif 44 <= gtile <= 47:
                                emit_reduce_chunk(s + 1, 1, gtile - 44)
                            elif 48 <= gtile <= 51:
                                emit_router_stage(s + 1, gtile - 48)
                        N = NT if t < 26 else 324
                        n0 = t * NT
                        tsbs = []
                        for ep in range(2):
                            cps = pp.tile([128, N], F32, tag=f"pc{ep}", bufs=2)
                            base = ep * 512
                            nc.tensor.matmul(cps[:, :], wk_sb[:, base:base + 128],
                                             regA[:, n0:n0 + N], start=True, stop=False)
                            nc.tensor.matmul(cps[:, :], wk_sb[:, base + 128:base + 256],
                                             regA[:, n0 + 162:n0 + 162 + N],
                                             start=False, stop=False)
                            nc.tensor.matmul(cps[:, :], wk_sb[:, base + 256:base + 384],
                                             regA[:, n0 + 324:n0 + 324 + N],
                                             start=False, stop=False)
                            nc.tensor.matmul(cps[:, :], wk_sb[:, base + 384:base + 512],
                                             regB[:, n0:n0 + N], start=False, stop=False)
                            nc.tensor.matmul(cps[:, :], wks_sb[:, ep * 128:ep * 128 + 128],
                                             regB[:, n0 + 324:n0 + 324 + N],
                                             start=False, stop=True)
                            tsb = wp.tile([128, N], BF16, tag=f"t{ep}", bufs=17)
                            nc.scalar.activation(tsb[:, :], cps[:, :], AF.Silu,
                                                 bias=bnb_sb[:, ep:ep + 1])
                            tsbs.append(tsb)
                        pending.append((tsbs, N, h, t, s))
                        if 0 <= gtile - ROUTER_G <= 3:
                            emit_router_stage(s, gtile - ROUTER_G)
                        if s > 0 or gtile > ROUTER_G + 3:
                            k = 0
                            while len(pending) > 2 and k < 3:
                                emit_mix(pending.pop(0))
                                k += 1
                # drain this image's tail so bounce groups stay h-aligned
                while pending:
                    emit_mix(pending.pop(0))

    nc.compile()
    return nc


def _prep_weights(fc1_w, fc2_w, fc2_b, conv_w, bn_gamma, bn_beta, bn_mean, bn_var):
    scale = bn_gamma / np.sqrt(bn_var + BN_EPS)            # [E, C]
    shift = bn_beta - bn_mean * scale                      # [E, C]
    ws = conv_w * scale[:, :, None, None, None]            # [E, Co, Ci, 3, 3]

    # paired-tap lhsT blocks: [K=128 (2 taps x 64 ci), M=128 (2 experts x 64 co)]
    groups = [((0, 0), (0, 1)), ((1, 0), (1, 1)), ((2, 0), (2, 1)), ((0, 2), (1, 2))]
    wk = np.zeros((128, 1024), np.float32)
    for ep in range(2):
        for g, (ta, tb) in enumerate(groups):
            blk = np.stack([ws[:, :, :, ta[0], ta[1]], ws[:, :, :, tb[0], tb[1]]])
            blk = blk[:, 2 * ep:2 * ep + 2]                # [j, le, Co, Ci]
            lhsT = blk.transpose(0, 3, 1, 2).reshape(128, 128)
            wk[:, (ep * 4 + g) * 128:(ep * 4 + g + 1) * 128] = lhsT
    wks = np.zeros((128, 256), np.float32)                 # rows 64-127 stay zero
    s22 = ws[:, :, :, 2, 2]                                # [E, Co, Ci]
    for ep in range(2):
        blk = s22[2 * ep:2 * ep + 2]                       # [le, Co, Ci]
        wks[0:64, ep * 128:(ep + 1) * 128] = blk.transpose(2, 0, 1).reshape(64, 128)

    # packed bf16 consts: wk | wks | fc1t | fc2t
    cbf = np.zeros((128, 1300), np.float32)
    cbf[:, 0:1024] = wk
    cbf[:, 1024:1280] = wks
    cbf[0:64, 1280:1296] = fc1_w.T / float(H * W)          # [64, 16]
    cbf[0:16, 1296:1300] = fc2_w.T                         # [16, 4]

    # packed f32 consts: idc | bnb | fc2b
    cf32 = np.zeros((128, 131), np.float32)
    cf32[:, 0:128] = np.concatenate([
        (np.arange(128)[:, None] % 64 == np.arange(64)[None, :]),
        np.zeros((128, 64), bool)], axis=1).astype(np.float32)
    cf32[:, 128] = np.concatenate([shift[0], shift[1]])
    cf32[:, 129] = np.concatenate([shift[2], shift[3]])
    cf32[0:4, 130] = fc2_b

    return {"cbf": cbf.astype(BF16_NP), "cf32": cf32}


def kernel(x, fc1_w, fc2_w, fc2_b, conv_w, bn_gamma, bn_beta, bn_mean, bn_var):
    from concourse.bass_utils import run_bass_kernel_spmd

    x = np.asarray(x, np.float32)
    reps = int(os.environ.get("BASS_KERNEL_REPS", "1"))
    key = f"nc{reps}"
    if key not in _CACHE:
        _CACHE[key] = _build_program(reps)
    nc = _CACHE[key]

    wmap = _prep_weights(np.asarray(fc1_w, np.float32), np.asarray(fc2_w, np.float32),
                         np.asarray(fc2_b, np.float32), np.asarray(conv_w, np.float32),
                         np.asarray(bn_gamma, np.float32), np.asarray(bn_beta, np.float32),
                         np.asarray(bn_mean, np.float32), np.asarray(bn_var, np.float32))

    # host-side zero-pad + bf16 cast into the flat 162x162 (+margin) layout,
    # then 4 tap-shifted copies (shifts 0/1/2/164) so each SBUF region loads
    # as one full-width 128-partition DMA
    xp = np.zeros((B, 4, C, LHOST), BF16_NP)
    xpad = xp[:, 0, :, :LP].reshape(B, C, H + 2, WP)
    xpad[:, :, 1:H + 1, 1:W + 1] = x.astype(BF16_NP)
    for j, sft in ((1, 1), (2, 2), (3, 164)):
        xp[:, j, :, :LHOST - sft] = xp[:, 0, :, sft:]

    in_maps = []
    for c in range(NCORES):
        m = dict(wmap)
        m["xp"] = xp[c * IMG_PER_CORE:(c + 1) * IMG_PER_CORE]
        in_maps.append(m)

    trace = bool(int(os.environ.get("BASS_KERNEL_TRACE", "0")))
    res = run_bass_kernel_spmd(nc, in_maps, list(range(NCORES)), trace=trace)
    _CACHE["last_results"] = res
    return np.concatenate([res.results[c]["out"] for c in range(NCORES)], axis=0)
